# revision 20
# baseline (speedup 1.0000x reference)
"""Trainium2 Bass kernel for nn_CIFARDiffusionLayer (5394478923805).

The reference module is LINEAR in u:
  - every tridiagonal ADI solve has batch-independent coefficients
    (built from the tiny [C,32,32] parameter maps), and
  - einsum('cc,bchw->bchw', coupling, u) with the repeated index is a
    per-channel diagonal scale.
So the whole 4-step loop collapses, per channel, to one dense [1024,1024]
matrix L_c acting on flattened 32x32 images:  out[b,c] = L_c @ vec(u[b,c]).
L_c is built on host in float64 by pushing the 1024 basis vectors through the
exact reference recurrences (including the EPS fudge).  In 128x128 chunk
blocks L_c is block-PENTAdiagonal to fp32 precision (|i-j|>2 blocks are ~0),
so the device kernel runs a banded block matmul — a single data-parallel pass
over u (one HBM read + one write = the memory roofline):

per 128-batch tile (per core, batch-sharded 8 ways):
  DMA u_nat[128b, 3072] (fp32r, pre-rounded on host)
  -> PE-transpose 24 [128,128] blocks to pixel-major (fp32r, 1.5 cyc/row)
  -> fp32r matmuls, data stationary / operator moving, N=512 so the per-MM
     LDWEIGHTS (~173ns) hides under the 227ns stream; accumulate the 6
     in-band k-chunks per output half; output emerges batch-major in PSUM
  -> ACT copy to SBUF, DMA out.

fp32r = fp32 rounded to 11 explicit mantissa bits; full-rate PE mode.
End-to-end error vs the reference is ~2.6e-4 of output absmax.
"""
import os
from contextlib import ExitStack

import numpy as np

DT = 0.15
DX = 1.0
NUM_STEPS = 4
EPS = 1e-6
S = 32
C = 3
PIX = S * S          # 1024
KC = PIX // 128      # 8 k-chunks per channel
ROW = C * PIX        # 3072 floats per batch
B_TOTAL = 16384
N_CORES = 8
B_CORE = B_TOTAL // N_CORES
BAND = int(os.environ.get("KERNEL_BAND", "2"))  # block band half-width


def _klist(h):
    """In-band k-chunks for output half h (m-chunks 4h..4h+3)."""
    return list(range(max(0, 4 * h - BAND), min(KC, 4 * h + 3 + BAND + 1)))


def _slices(h):
    """Tight column ranges per in-band k for half h: [(k, col_start, col_end)].

    Only m-chunks within BAND of k are nonzero; ranges padded to >=256 cols
    (fp32r matmul needs a moving dim >= 256 for full rate).  Accumulation with
    per-k partial column ranges is safe: the start=True matmul clears the whole
    PSUM bank's has_written bits, so each element's first writer overwrites.
    """
    res = []
    for k in _klist(h):
        mlo = max(4 * h, k - BAND)
        mhi = min(4 * h + 4, k + 1 + BAND)
        cs = (mlo - 4 * h) * 128
        ce = (mhi - 4 * h) * 128
        while ce - cs < 256:
            if cs >= 128:
                cs -= 128
            elif ce <= 384:
                ce += 128
        res.append((k, cs, ce))
    return res


def _wtot(h):
    return sum(ce - cs for _, cs, ce in _slices(h))


_CACHE = {}
LAST_RESULTS = None  # BassKernelResults of the most recent run (for test.py)


# ----------------------------- host-side operator ---------------------------

def _smooth3(m, axis):
    p = np.concatenate([m.take([0], axis=axis), m, m.take([-1], axis=axis)],
                       axis=axis)
    n = m.shape[axis]
    sl = lambda i: p.take(range(i, i + n), axis=axis)
    return (sl(0) + sl(1) + sl(2)) / 3.0


def _thomas_matrix(a, b, c):
    """Exact linear map of the reference thomas() for one N-system, as [N,N]."""
    N = a.shape[0]
    d = np.eye(N, dtype=np.float64)
    cp = 0.0
    dp = np.zeros(N, dtype=np.float64)
    cs = np.zeros(N, dtype=np.float64)
    ds = np.zeros((N, N), dtype=np.float64)
    for i in range(N):
        denom = b[i] - a[i] * cp + EPS
        cn = c[i] / denom
        dn = (d[i] - a[i] * dp) / denom
        cs[i] = cn
        ds[i] = dn
        cp, dp = cn, dn
    cs[N - 1] = 0.0
    x = np.zeros((N, N), dtype=np.float64)
    xn = np.zeros(N, dtype=np.float64)
    for i in range(N - 1, -1, -1):
        x[i] = ds[i] - cs[i] * xn
        xn = x[i]
    return x


def _solve_matrices(coeff_smooth, dt):
    coeff = coeff_smooth * dt / (DX ** 2)
    a = -coeff
    c = -coeff
    b = 1.0 + 2.0 * coeff
    b = b.copy()
    b[..., 0] = 1.0 + coeff[..., 0]
    b[..., -1] = 1.0 + coeff[..., -1]
    Cn, K, N = a.shape
    out = np.zeros((Cn, K, N, N), dtype=np.float64)
    for ci in range(Cn):
        for k in range(K):
            out[ci, k] = _thomas_matrix(a[ci, k], b[ci, k], c[ci, k])
    return out


def _build_operator(alpha_base, beta_base, alpha_time_coeff, beta_time_coeff,
                    channel_coupling):
    """[C, 1024, 1024] float64: out_vec = L[c] @ u_vec (h*32+w order)."""
    ab = alpha_base.astype(np.float64)
    bb = beta_base.astype(np.float64)
    at = alpha_time_coeff.astype(np.float64)
    bt = beta_time_coeff.astype(np.float64)
    diag = np.diag(channel_coupling.astype(np.float64))

    M = np.broadcast_to(np.eye(PIX, dtype=np.float64).reshape(S, S, PIX),
                        (C, S, S, PIX)).copy()
    t = 0.0
    for _ in range(NUM_STEPS):
        alpha = np.maximum(ab + at * t, EPS)
        beta = np.maximum(bb + bt * t, EPS)
        Sx = _solve_matrices(_smooth3(alpha, axis=2), DT / 2)        # [C,H,w',w]
        bsm = _smooth3(beta, axis=1)
        Sy = _solve_matrices(np.transpose(bsm, (0, 2, 1)), DT)       # [C,W,h',h]
        M = np.einsum('chvw,chwK->chvK', Sx, M)
        M = np.einsum('cwuh,chwK->cuwK', Sy, M)
        M = np.einsum('chvw,chwK->chvK', Sx, M)
        M = M * diag[:, None, None, None]
        t += DT
    return M.reshape(C, PIX, PIX)


def _round_fp32r(x):
    """Round f32 to fp32r: 11 explicit mantissa bits (RNE), low 12 bits zero."""
    b = np.ascontiguousarray(x, dtype=np.float32).view(np.uint32)
    lsb = (b >> np.uint32(12)) & np.uint32(1)
    r = (b + np.uint32(0x7FF) + lsb) & np.uint32(0xFFFFF000)
    return r.view(np.float32)


# ----------------------------- device program -------------------------------

def _build_program(nc, u_ap, w_ap, id_ap, out_ap, b_per_core):
    import concourse.tile as tile
    from concourse import mybir
    F32 = mybir.dt.float32
    F32R = mybir.dt.float32r
    ntiles = b_per_core // 128

    with tile.TileContext(nc) as tc, ExitStack() as ctx:
        const_pool = ctx.enter_context(tc.tile_pool(name="const", bufs=1))
        w_pool = ctx.enter_context(tc.tile_pool(name="w", bufs=1))
        u_pool = ctx.enter_context(tc.tile_pool(name="u", bufs=4))
        ut_pool = ctx.enter_context(tc.tile_pool(name="ut", bufs=4))
        out_pool = ctx.enter_context(tc.tile_pool(name="out", bufs=3))
        pst_pool = ctx.enter_context(tc.tile_pool(name="pst", bufs=3,
                                                  space="PSUM"))
        psm_pool = ctx.enter_context(tc.tile_pool(name="psm", bufs=5,
                                                  space="PSUM"))

        ident = const_pool.tile([128, 128], F32R)
        nc.sync.dma_start(out=ident[:], in_=id_ap[:])

        # Prologue DMA: operator slices stream on the sync queue while u1/u2
        # stream concurrently on the scalar HWDGE queue, so the PE can run
        # tile-0..2 transposes while W is still in flight.  Output stores go
        # on the sync queue (u has 4-deep prefetch slack there) keeping the
        # ACT queue free for the PSUM-draining copies.
        wtot = _wtot(0)
        wt = [[None] * 2 for _ in range(C)]
        u_tiles = {}
        PRO = min(3, ntiles)
        # u0 alone on the sync queue so it lands first; u1/u2 on the scalar
        # queue race with the operator stream.
        u_tiles[0] = u_pool.tile([128, ROW], F32R, tag="u_nat", name="u_nat")
        nc.sync.dma_start(out=u_tiles[0][:], in_=u_ap[0:128, :])
        for it in range(1, PRO):
            u_tiles[it] = u_pool.tile([128, ROW], F32R, tag="u_nat",
                                      name="u_nat")
            nc.scalar.dma_start(out=u_tiles[it][:],
                                in_=u_ap[it * 128:(it + 1) * 128, :])
        for c in range(C):
            for h in range(2):
                t = w_pool.tile([128, wtot], F32R, tag=f"w{c}_{h}")
                nc.sync.dma_start(out=t[:], in_=w_ap[c, h])
                wt[c][h] = t

        def emit_transposes(u_nat):
            groups = []
            for g in range(ROW // 512):
                pt = pst_pool.tile([128, 512], F32R, tag="pst", name="pt")
                for j in range(4):
                    blk = g * 4 + j
                    nc.tensor.transpose(
                        pt[:, j * 128:(j + 1) * 128],
                        u_nat[:, blk * 128:(blk + 1) * 128], ident[:])
                st = ut_pool.tile([128, 512], F32R, tag=f"utg{g}",
                                  name="utg")
                nc.vector.tensor_copy(st[:], pt[:])  # exact: values are fp32r
                groups.append(st)
            return [[groups[2 * c + k // 4][:, (k % 4) * 128:(k % 4 + 1) * 128]
                     for k in range(KC)] for c in range(C)]

        def emit_matmuls(it, ut):
            out_nat = out_pool.tile([128, ROW], F32, name="out_nat")
            for c in range(C):
                for h in range(2):
                    sl = _slices(h)
                    ps = psm_pool.tile([128, 512], F32, tag="psm", name="ps")
                    off = 0
                    for i, (k, cs, ce) in enumerate(sl):
                        nc.tensor.matmul(
                            ps[:, cs:ce], lhsT=ut[c][k],
                            rhs=wt[c][h][:, off:off + (ce - cs)],
                            start=(i == 0), stop=(i == len(sl) - 1))
                        off += ce - cs
                    nc.scalar.copy(
                        out_nat[:, c * PIX + h * 512:c * PIX + (h + 1) * 512],
                        ps[:])
                # store each channel as soon as its copies land; issued on
                # the sync queue so ring backpressure never blocks the ACT
                # copies that free PSUM banks
                nc.sync.dma_start(
                    out=out_ap[it * 128:(it + 1) * 128,
                               c * PIX:(c + 1) * PIX],
                    in_=out_nat[:, c * PIX:(c + 1) * PIX])

        # Prologue: transposes for the first tiles run while W streams in.
        pro_ut = [emit_transposes(u_tiles[it]) for it in range(PRO)]
        for it in range(PRO):
            emit_matmuls(it, pro_ut[it])

        for it in range(PRO, ntiles):
            u_nat = u_pool.tile([128, ROW], F32R, tag="u_nat", name="u_nat")
            nc.sync.dma_start(out=u_nat[:],
                              in_=u_ap[it * 128:(it + 1) * 128, :])
            emit_matmuls(it, emit_transposes(u_nat))


def _get_nc():
    if "nc" in _CACHE:
        return _CACHE["nc"]
    from concourse import bacc, mybir
    # num_devices=1: the 8 cores are pure SPMD replicas with no collectives,
    # so skip the cross-core EVSEM butterfly in the kernel pre/postamble.
    nd = int(os.environ.get("KERNEL_ND", "1"))
    nc = bacc.Bacc("TRN2", target_bir_lowering=False, debug=False,
                   num_devices=nd)
    F32 = mybir.dt.float32
    F32R = mybir.dt.float32r
    u_ap = nc.dram_tensor("u", [B_CORE, ROW], F32R, kind="ExternalInput").ap()
    w_ap = nc.dram_tensor("w", [C, 2, 128, _wtot(0)], F32R,
                          kind="ExternalInput").ap()
    id_ap = nc.dram_tensor("ident", [128, 128], F32R,
                           kind="ExternalInput").ap()
    out_ap = nc.dram_tensor("out", [B_CORE, ROW], F32,
                            kind="ExternalOutput").ap()
    _build_program(nc, u_ap, w_ap, id_ap, out_ap, B_CORE)
    nc.compile()
    _CACHE["nc"] = nc
    return nc


def _inject_ntff_hook():
    import sys, types
    try:
        import antenv.axon_hooks  # noqa: F401
        return
    except ImportError:
        pass
    from trn_agent_boot.trn_boot import _ntff_profile_via_ctypes
    hook = _ntff_profile_via_ctypes('/opt/axon/libaxon_pjrt.so')
    mod = types.ModuleType('antenv.axon_hooks')
    _state = {'hook': hook}
    mod.get_axon_ntff_profile_hook = lambda: _state['hook']
    mod.set_axon_ntff_profile_hook = lambda h: _state.update(hook=h)
    sys.modules['antenv.axon_hooks'] = mod
    import antenv
    antenv.axon_hooks = mod


# ----------------------------- entry point ----------------------------------

def kernel(u, alpha_base, beta_base, alpha_time_coeff, beta_time_coeff,
           channel_coupling):
    global LAST_RESULTS
    u = np.asarray(u, dtype=np.float32)
    assert u.shape == (B_TOTAL, C, S, S), u.shape

    L = _build_operator(np.asarray(alpha_base), np.asarray(beta_base),
                        np.asarray(alpha_time_coeff),
                        np.asarray(beta_time_coeff),
                        np.asarray(channel_coupling))
    # tight-packed banded moving-operand slices, concatenated along free dim
    wtot = _wtot(0)
    w = np.zeros((C, 2, 128, wtot), dtype=np.float32)
    LT = L.transpose(0, 2, 1).astype(np.float32)  # [c, kpix, npix]
    for h in range(2):
        off = 0
        for k, cs, ce in _slices(h):
            w[:, h, :, off:off + (ce - cs)] = \
                LT[:, k * 128:(k + 1) * 128, 512 * h + cs:512 * h + ce]
            off += ce - cs
    w = _round_fp32r(w)
    ident = _round_fp32r(np.eye(128, dtype=np.float32))

    nc = _get_nc()
    from concourse import bass_utils

    u2 = _round_fp32r(u.reshape(B_TOTAL, ROW))
    in_maps = [{"u": u2[i * B_CORE:(i + 1) * B_CORE], "w": w, "ident": ident}
               for i in range(N_CORES)]

    trace = os.environ.get("KERNEL_TRACE", "") == "1"
    kw = {}
    if trace:
        _inject_ntff_hook()
        bass_utils.upload_artifacts = lambda tmpdir: tmpdir
        kw = dict(trace=True, tmpdir=os.environ.get("KERNEL_TRACE_DIR"))

    try:
        res = bass_utils.run_bass_kernel_spmd(
            nc, in_maps, core_ids=list(range(N_CORES)), **kw)
    except Exception:
        # the axon-tunneled devices occasionally report a transient
        # NRT_EXEC_UNIT_UNRECOVERABLE; one clean retry recovers
        import time
        time.sleep(5)
        res = bass_utils.run_bass_kernel_spmd(
            nc, in_maps, core_ids=list(range(N_CORES)))
    LAST_RESULTS = res

    out = np.concatenate([r["out"] for r in res.results], axis=0)
    return out.reshape(B_TOTAL, C, S, S)


# revision 25
# speedup vs baseline: 1.0106x; 1.0106x over previous
"""Trainium2 Bass kernel for nn_CIFARDiffusionLayer (5394478923805).

The reference module is LINEAR in u:
  - every tridiagonal ADI solve has batch-independent coefficients
    (built from the tiny [C,32,32] parameter maps), and
  - einsum('cc,bchw->bchw', coupling, u) with the repeated index is a
    per-channel diagonal scale.
So the whole 4-step loop collapses, per channel, to one dense [1024,1024]
matrix L_c acting on flattened 32x32 images:  out[b,c] = L_c @ vec(u[b,c]).
L_c is built on host in float64 by pushing the 1024 basis vectors through the
exact reference recurrences (including the EPS fudge).  In 128x128 chunk
blocks L_c is block-PENTAdiagonal to fp32 precision (|i-j|>2 blocks are ~0),
so the device kernel runs a banded block matmul — a single data-parallel pass
over u (one HBM read + one write = the memory roofline):

per 128-batch tile (per core, batch-sharded 8 ways):
  DMA u_nat[128b, 3072] (fp32r, pre-rounded on host)
  -> PE-transpose 24 [128,128] blocks to pixel-major (fp32r, 1.5 cyc/row)
  -> fp32r matmuls, data stationary / operator moving, N=512 so the per-MM
     LDWEIGHTS (~173ns) hides under the 227ns stream; accumulate the 6
     in-band k-chunks per output half; output emerges batch-major in PSUM
  -> ACT copy to SBUF, DMA out.

fp32r = fp32 rounded to 11 explicit mantissa bits; full-rate PE mode.
End-to-end error vs the reference is ~2.6e-4 of output absmax.
"""
import os
from contextlib import ExitStack

import numpy as np

DT = 0.15
DX = 1.0
NUM_STEPS = 4
EPS = 1e-6
S = 32
C = 3
PIX = S * S          # 1024
KC = PIX // 128      # 8 k-chunks per channel
ROW = C * PIX        # 3072 floats per batch
B_TOTAL = 16384
N_CORES = 8
B_CORE = B_TOTAL // N_CORES
BAND = int(os.environ.get("KERNEL_BAND", "2"))  # block band half-width


def _klist(h):
    """In-band k-chunks for output half h (m-chunks 4h..4h+3)."""
    return list(range(max(0, 4 * h - BAND), min(KC, 4 * h + 3 + BAND + 1)))


def _slices(h):
    """Tight column ranges per in-band k for half h: [(k, col_start, col_end)].

    Only m-chunks within BAND of k are nonzero; ranges padded to >=256 cols
    (fp32r matmul needs a moving dim >= 256 for full rate).  Accumulation with
    per-k partial column ranges is safe: the start=True matmul clears the whole
    PSUM bank's has_written bits, so each element's first writer overwrites.
    """
    res = []
    for k in _klist(h):
        mlo = max(4 * h, k - BAND)
        mhi = min(4 * h + 4, k + 1 + BAND)
        cs = (mlo - 4 * h) * 128
        ce = (mhi - 4 * h) * 128
        while ce - cs < 256:
            if cs >= 128:
                cs -= 128
            elif ce <= 384:
                ce += 128
        res.append((k, cs, ce))
    return res


def _wtot(h):
    return sum(ce - cs for _, cs, ce in _slices(h))


_CACHE = {}
LAST_RESULTS = None  # BassKernelResults of the most recent run (for test.py)


# ----------------------------- host-side operator ---------------------------

def _smooth3(m, axis):
    p = np.concatenate([m.take([0], axis=axis), m, m.take([-1], axis=axis)],
                       axis=axis)
    n = m.shape[axis]
    sl = lambda i: p.take(range(i, i + n), axis=axis)
    return (sl(0) + sl(1) + sl(2)) / 3.0


def _thomas_matrix(a, b, c):
    """Exact linear map of the reference thomas() for one N-system, as [N,N]."""
    N = a.shape[0]
    d = np.eye(N, dtype=np.float64)
    cp = 0.0
    dp = np.zeros(N, dtype=np.float64)
    cs = np.zeros(N, dtype=np.float64)
    ds = np.zeros((N, N), dtype=np.float64)
    for i in range(N):
        denom = b[i] - a[i] * cp + EPS
        cn = c[i] / denom
        dn = (d[i] - a[i] * dp) / denom
        cs[i] = cn
        ds[i] = dn
        cp, dp = cn, dn
    cs[N - 1] = 0.0
    x = np.zeros((N, N), dtype=np.float64)
    xn = np.zeros(N, dtype=np.float64)
    for i in range(N - 1, -1, -1):
        x[i] = ds[i] - cs[i] * xn
        xn = x[i]
    return x


def _solve_matrices(coeff_smooth, dt):
    coeff = coeff_smooth * dt / (DX ** 2)
    a = -coeff
    c = -coeff
    b = 1.0 + 2.0 * coeff
    b = b.copy()
    b[..., 0] = 1.0 + coeff[..., 0]
    b[..., -1] = 1.0 + coeff[..., -1]
    Cn, K, N = a.shape
    out = np.zeros((Cn, K, N, N), dtype=np.float64)
    for ci in range(Cn):
        for k in range(K):
            out[ci, k] = _thomas_matrix(a[ci, k], b[ci, k], c[ci, k])
    return out


def _build_operator(alpha_base, beta_base, alpha_time_coeff, beta_time_coeff,
                    channel_coupling):
    """[C, 1024, 1024] float64: out_vec = L[c] @ u_vec (h*32+w order)."""
    ab = alpha_base.astype(np.float64)
    bb = beta_base.astype(np.float64)
    at = alpha_time_coeff.astype(np.float64)
    bt = beta_time_coeff.astype(np.float64)
    diag = np.diag(channel_coupling.astype(np.float64))

    M = np.broadcast_to(np.eye(PIX, dtype=np.float64).reshape(S, S, PIX),
                        (C, S, S, PIX)).copy()
    t = 0.0
    for _ in range(NUM_STEPS):
        alpha = np.maximum(ab + at * t, EPS)
        beta = np.maximum(bb + bt * t, EPS)
        Sx = _solve_matrices(_smooth3(alpha, axis=2), DT / 2)        # [C,H,w',w]
        bsm = _smooth3(beta, axis=1)
        Sy = _solve_matrices(np.transpose(bsm, (0, 2, 1)), DT)       # [C,W,h',h]
        M = np.einsum('chvw,chwK->chvK', Sx, M)
        M = np.einsum('cwuh,chwK->cuwK', Sy, M)
        M = np.einsum('chvw,chwK->chvK', Sx, M)
        M = M * diag[:, None, None, None]
        t += DT
    return M.reshape(C, PIX, PIX)


def _round_fp32r(x):
    """Round f32 to fp32r: 11 explicit mantissa bits (RNE), low 12 bits zero."""
    b = np.ascontiguousarray(x, dtype=np.float32).view(np.uint32)
    lsb = (b >> np.uint32(12)) & np.uint32(1)
    r = (b + np.uint32(0x7FF) + lsb) & np.uint32(0xFFFFF000)
    return r.view(np.float32)


# ----------------------------- device program -------------------------------

def _build_program(nc, u_ap, w_ap, id_ap, out_ap, b_per_core):
    import concourse.tile as tile
    from concourse import mybir
    F32 = mybir.dt.float32
    F32R = mybir.dt.float32r
    ntiles = b_per_core // 128

    with tile.TileContext(nc) as tc, ExitStack() as ctx:
        const_pool = ctx.enter_context(tc.tile_pool(name="const", bufs=1))
        w_pool = ctx.enter_context(tc.tile_pool(name="w", bufs=1))
        u_pool = ctx.enter_context(tc.tile_pool(name="u", bufs=4))
        ut_pool = ctx.enter_context(tc.tile_pool(name="ut", bufs=4))
        out_pool = ctx.enter_context(tc.tile_pool(name="out", bufs=3))
        pst_pool = ctx.enter_context(tc.tile_pool(name="pst", bufs=3,
                                                  space="PSUM"))
        psm_pool = ctx.enter_context(tc.tile_pool(name="psm", bufs=5,
                                                  space="PSUM"))

        ident = const_pool.tile([128, 128], F32R)
        nc.sync.dma_start(out=ident[:], in_=id_ap[:])

        # HAM warm-up: ~40 throwaway transposes of the identity keep the PE
        # clock-gate at 8/8 through the W-load window, so the first real
        # tiles don't run at the cold 1.2 GHz half rate.
        for wi in range(40):
            wp = pst_pool.tile([128, 128], F32R, tag="pst", name="warm")
            nc.tensor.transpose(wp[:], ident[:], ident[:])

        # Prologue DMA: operator slices stream on the sync queue while u1/u2
        # stream concurrently on the scalar HWDGE queue, so the PE can run
        # tile-0..2 transposes while W is still in flight.  Output stores go
        # on the sync queue (u has 4-deep prefetch slack there) keeping the
        # ACT queue free for the PSUM-draining copies.
        wtot = _wtot(0)
        wt = [[None] * 2 for _ in range(C)]
        u_tiles = {}
        PRO = min(3, ntiles)
        # u0 alone on the sync queue so it lands first; u1/u2 on the scalar
        # queue race with the operator stream.
        u_tiles[0] = u_pool.tile([128, ROW], F32R, tag="u_nat", name="u_nat")
        nc.sync.dma_start(out=u_tiles[0][:], in_=u_ap[0:128, :])
        for it in range(1, PRO):
            u_tiles[it] = u_pool.tile([128, ROW], F32R, tag="u_nat",
                                      name="u_nat")
            nc.scalar.dma_start(out=u_tiles[it][:],
                                in_=u_ap[it * 128:(it + 1) * 128, :])
        for c in range(C):
            for h in range(2):
                t = w_pool.tile([128, wtot], F32R, tag=f"w{c}_{h}")
                nc.sync.dma_start(out=t[:], in_=w_ap[c, h])
                wt[c][h] = t

        def emit_transposes(u_nat):
            groups = []
            for g in range(ROW // 512):
                pt = pst_pool.tile([128, 512], F32R, tag="pst", name="pt")
                for j in range(4):
                    blk = g * 4 + j
                    nc.tensor.transpose(
                        pt[:, j * 128:(j + 1) * 128],
                        u_nat[:, blk * 128:(blk + 1) * 128], ident[:])
                st = ut_pool.tile([128, 512], F32R, tag=f"utg{g}",
                                  name="utg")
                nc.vector.tensor_copy(st[:], pt[:])  # exact: values are fp32r
                groups.append(st)
            return [[groups[2 * c + k // 4][:, (k % 4) * 128:(k % 4 + 1) * 128]
                     for k in range(KC)] for c in range(C)]

        def emit_matmuls(it, ut):
            out_nat = out_pool.tile([128, ROW], F32, name="out_nat")
            for c in range(C):
                for h in range(2):
                    sl = _slices(h)
                    ps = psm_pool.tile([128, 512], F32, tag="psm", name="ps")
                    off = 0
                    for i, (k, cs, ce) in enumerate(sl):
                        nc.tensor.matmul(
                            ps[:, cs:ce], lhsT=ut[c][k],
                            rhs=wt[c][h][:, off:off + (ce - cs)],
                            start=(i == 0), stop=(i == len(sl) - 1))
                        off += ce - cs
                    nc.scalar.copy(
                        out_nat[:, c * PIX + h * 512:c * PIX + (h + 1) * 512],
                        ps[:])
                # store each channel as soon as its copies land; issued on
                # the sync queue so ring backpressure never blocks the ACT
                # copies that free PSUM banks
                nc.sync.dma_start(
                    out=out_ap[it * 128:(it + 1) * 128,
                               c * PIX:(c + 1) * PIX],
                    in_=out_nat[:, c * PIX:(c + 1) * PIX])

        # Prologue: transposes for the first tiles run while W streams in.
        pro_ut = [emit_transposes(u_tiles[it]) for it in range(PRO)]
        for it in range(PRO):
            emit_matmuls(it, pro_ut[it])

        for it in range(PRO, ntiles):
            u_nat = u_pool.tile([128, ROW], F32R, tag="u_nat", name="u_nat")
            nc.sync.dma_start(out=u_nat[:],
                              in_=u_ap[it * 128:(it + 1) * 128, :])
            emit_matmuls(it, emit_transposes(u_nat))


def _get_nc():
    if "nc" in _CACHE:
        return _CACHE["nc"]
    from concourse import bacc, mybir
    # num_devices=1: the 8 cores are pure SPMD replicas with no collectives,
    # so skip the cross-core EVSEM butterfly in the kernel pre/postamble.
    nd = int(os.environ.get("KERNEL_ND", "1"))
    nc = bacc.Bacc("TRN2", target_bir_lowering=False, debug=False,
                   num_devices=nd)
    F32 = mybir.dt.float32
    F32R = mybir.dt.float32r
    u_ap = nc.dram_tensor("u", [B_CORE, ROW], F32R, kind="ExternalInput").ap()
    w_ap = nc.dram_tensor("w", [C, 2, 128, _wtot(0)], F32R,
                          kind="ExternalInput").ap()
    id_ap = nc.dram_tensor("ident", [128, 128], F32R,
                           kind="ExternalInput").ap()
    out_ap = nc.dram_tensor("out", [B_CORE, ROW], F32,
                            kind="ExternalOutput").ap()
    _build_program(nc, u_ap, w_ap, id_ap, out_ap, B_CORE)
    nc.compile()
    _CACHE["nc"] = nc
    return nc


def _inject_ntff_hook():
    import sys, types
    try:
        import antenv.axon_hooks  # noqa: F401
        return
    except ImportError:
        pass
    from trn_agent_boot.trn_boot import _ntff_profile_via_ctypes
    hook = _ntff_profile_via_ctypes('/opt/axon/libaxon_pjrt.so')
    mod = types.ModuleType('antenv.axon_hooks')
    _state = {'hook': hook}
    mod.get_axon_ntff_profile_hook = lambda: _state['hook']
    mod.set_axon_ntff_profile_hook = lambda h: _state.update(hook=h)
    sys.modules['antenv.axon_hooks'] = mod
    import antenv
    antenv.axon_hooks = mod


# ----------------------------- entry point ----------------------------------

def kernel(u, alpha_base, beta_base, alpha_time_coeff, beta_time_coeff,
           channel_coupling):
    global LAST_RESULTS
    u = np.asarray(u, dtype=np.float32)
    assert u.shape == (B_TOTAL, C, S, S), u.shape

    L = _build_operator(np.asarray(alpha_base), np.asarray(beta_base),
                        np.asarray(alpha_time_coeff),
                        np.asarray(beta_time_coeff),
                        np.asarray(channel_coupling))
    # tight-packed banded moving-operand slices, concatenated along free dim
    wtot = _wtot(0)
    w = np.zeros((C, 2, 128, wtot), dtype=np.float32)
    LT = L.transpose(0, 2, 1).astype(np.float32)  # [c, kpix, npix]
    for h in range(2):
        off = 0
        for k, cs, ce in _slices(h):
            w[:, h, :, off:off + (ce - cs)] = \
                LT[:, k * 128:(k + 1) * 128, 512 * h + cs:512 * h + ce]
            off += ce - cs
    w = _round_fp32r(w)
    ident = _round_fp32r(np.eye(128, dtype=np.float32))

    nc = _get_nc()
    from concourse import bass_utils

    u2 = _round_fp32r(u.reshape(B_TOTAL, ROW))
    in_maps = [{"u": u2[i * B_CORE:(i + 1) * B_CORE], "w": w, "ident": ident}
               for i in range(N_CORES)]

    trace = os.environ.get("KERNEL_TRACE", "") == "1"
    kw = {}
    if trace:
        _inject_ntff_hook()
        bass_utils.upload_artifacts = lambda tmpdir: tmpdir
        kw = dict(trace=True, tmpdir=os.environ.get("KERNEL_TRACE_DIR"))

    try:
        res = bass_utils.run_bass_kernel_spmd(
            nc, in_maps, core_ids=list(range(N_CORES)), **kw)
    except Exception:
        # the axon-tunneled devices occasionally report a transient
        # NRT_EXEC_UNIT_UNRECOVERABLE; one clean retry recovers
        import time
        time.sleep(5)
        res = bass_utils.run_bass_kernel_spmd(
            nc, in_maps, core_ids=list(range(N_CORES)))
    LAST_RESULTS = res

    out = np.concatenate([r["out"] for r in res.results], axis=0)
    return out.reshape(B_TOTAL, C, S, S)
